# revision 11
# baseline (speedup 1.0000x reference)
"""BertSelfAttention (relative_key_query position embeddings) on 8 TRN2 cores.

Problem: B=4, L=1024, DM=1024, H=16, D=64, MAXPOS=1024.
  q/k/v = hidden @ W.T + b  (per-head split)
  scores = q k^T + einsum(q, pe) + einsum(k, pe);  pe[l,r] = dist_emb[l-r+1023]
  probs = softmax(scores/8);  out = probs @ v

Sharding: core c handles batch b = c//2 and 8 heads (half) hh = c%2.
Per core the computation runs in "transposed score" orientation
(scoresT[r, l]): the PV matmul is transpose-free and the K-side
relative-position bias is a per-partition-shifted ("skewed") DMA from
Kd = k @ E^T.  The Q-side bias is extracted with the same skewed DMA in
natural orientation from Qrev = q @ Erev^T and folded into the score PSUM
with regular bf16 matmuls against an identity (out += b1n^T @ I).
Softmax uses no max-subtraction (scores are O(1) by construction) and
normalization is deferred past the PV matmul via an appended ones-column
on V; the row-sums ride along the final context transpose as a 65th
column so no scatter-DMAs are needed.

All matmul operands are bf16 (fp32 PSUM accumulation); phase 2 runs one
head-half at a time so 6 score/band PSUM banks + 2 context accumulators
fit the 8-bank budget with deep software pipelining (kd bands one r-tile
ahead, PV one r-tile behind) to keep the PE warm.
"""
import os
import sys

import numpy as np

if "/opt/trn_rl_repo" not in sys.path:
    sys.path.insert(0, "/opt/trn_rl_repo")

_PROGRAM = None
_LAST_RESULTS = None

# ---- fixed shapes ----
L = 1024          # sequence length
DM = 1024         # model dim
NH = 8            # heads per core
D = 64            # head dim
MT = 4            # m-tiles (head pairs) per core
KT = 8            # dm contraction tiles
LT = 8            # l (and r) tiles of 128
BAND = 1152       # stored band width per 128-row tile
ETW = 2048        # padded dist-emb width
CHUNKS = ((0, 512), (512, 512), (1024, 128))  # band psum chunking


def _t0(j):
    # band start for row-tile j: t = l - r + 1023 over full opposite dim
    return 896 - 128 * j


def _build_program():
    import concourse.bass as bass
    from concourse import bacc
    import concourse.mybir as mybir
    import concourse.tile as tile
    from concourse.masks import make_identity

    f32 = mybir.dt.float32
    bf16 = mybir.dt.bfloat16
    AluOp = mybir.AluOpType
    Act = mybir.ActivationFunctionType

    nc = bacc.Bacc("TRN2", target_bir_lowering=False, debug=False)

    hidT = nc.dram_tensor("hidT", [DM, L], bf16, kind="ExternalInput")
    wqT = nc.dram_tensor("wqT", [DM, 512], bf16, kind="ExternalInput")
    wkT = nc.dram_tensor("wkT", [DM, 512], bf16, kind="ExternalInput")
    wvT = nc.dram_tensor("wvT", [DM, 512], bf16, kind="ExternalInput")
    bq2 = nc.dram_tensor("bq2", [128, MT], f32, kind="ExternalInput")
    bk2 = nc.dram_tensor("bk2", [128, MT], f32, kind="ExternalInput")
    bvb = nc.dram_tensor("bvb", [128, 512], f32, kind="ExternalInput")
    ETd = nc.dram_tensor("ETd", [128, ETW], bf16, kind="ExternalInput")
    ERVd = nc.dram_tensor("ERVd", [128, ETW], bf16, kind="ExternalInput")
    outd = nc.dram_tensor("out", [L, 512], f32, kind="ExternalOutput")

    with tile.TileContext(nc) as tc:
        import contextlib
        stack = contextlib.ExitStack()
        with stack:
            persist = stack.enter_context(tc.tile_pool(name="persist", bufs=1))

            qT_sb = persist.tile([128, MT, L], bf16, name="qT_sb")
            kT_sb = persist.tile([128, MT, L], bf16, name="kT_sb")
            vaug = persist.tile([128, LT, MT, 130], bf16, name="vaug")
            ET_sb = persist.tile([128, ETW], bf16, name="ET_sb")
            ERV_sb = persist.tile([128, ETW], bf16, name="ERV_sb")
            ident32 = persist.tile([128, 128], f32, name="ident32")
            ident_bf = persist.tile([128, 128], bf16, name="ident_bf")
            bq_sb = persist.tile([128, MT], f32, name="bq_sb")
            bk_sb = persist.tile([128, MT], f32, name="bk_sb")
            bvb_sb = persist.tile([128, 512], f32, name="bvb_sb")

            # small/replicated tensors on the gpsimd (SWDGE) ring so the two
            # HWDGE rings are free for hid (sync) and weights (scalar)
            nc.gpsimd.dma_start(out=ET_sb, in_=ETd[:, :])
            nc.gpsimd.dma_start(out=ERV_sb, in_=ERVd[:, :])
            nc.gpsimd.dma_start(out=bq_sb, in_=bq2[:, :])
            nc.gpsimd.dma_start(out=bk_sb, in_=bk2[:, :])
            nc.gpsimd.dma_start(out=bvb_sb, in_=bvb[:, :])
            make_identity(nc, ident32)
            nc.vector.tensor_copy(ident_bf, ident32)

            # ones columns of vaug (col 64 and 129 of each [*,130] block)
            nc.vector.memset(vaug[:, :, :, 64:65], 1.0)
            nc.vector.memset(vaug[:, :, :, 129:130], 1.0)

            # ---------------- Stage A: projections ----------------
            with tc.tile_pool(name="stagea", bufs=1) as apool, \
                 tc.tile_pool(name="wpool", bufs=2) as wpool, \
                 tc.tile_pool(name="apsum", bufs=8, space="PSUM") as apsum:
                hid_sb = apool.tile([128, KT, L], bf16, name="hid_sb")
                for k in range(KT):
                    nc.sync.dma_start(out=hid_sb[:, k, :],
                                      in_=hidT[128 * k:128 * (k + 1), :])

                wq_sb = wpool.tile([128, KT, 512], bf16, name="wq_sb", tag="w")
                wk_sb = wpool.tile([128, KT, 512], bf16, name="wk_sb", tag="wk")
                wv_sb = wpool.tile([128, KT, 512], bf16, name="wv_sb", tag="wv")
                for w_sb, wdram in ((wq_sb, wqT), (wk_sb, wkT), (wv_sb, wvT)):
                    for k in range(KT):
                        nc.scalar.dma_start(out=w_sb[:, k, :],
                                            in_=wdram[128 * k:128 * (k + 1), :])

                # q: k-outer with 8 live accumulators — matmuls start as soon
                # as the first weight k-tile lands
                ps = {}
                for mt in range(MT):
                    for lc in range(2):
                        ps[(mt, lc)] = apsum.tile([128, 512], f32,
                                                  name=f"ps_q{mt}_{lc}",
                                                  tag="aps")
                for k in range(KT):
                    for mt in range(MT):
                        for lc in range(2):
                            nc.tensor.matmul(
                                ps[(mt, lc)],
                                wq_sb[:, k, 128 * mt:128 * (mt + 1)],
                                hid_sb[:, k, 512 * lc:512 * (lc + 1)],
                                start=(k == 0), stop=(k == KT - 1))
                for mt in range(MT):
                    for lc in range(2):
                        nc.scalar.activation(
                            out=qT_sb[:, mt, 512 * lc:512 * (lc + 1)],
                            in_=ps[(mt, lc)], func=Act.Identity,
                            bias=bq_sb[:, mt:mt + 1], scale=1.0)

                # k/v: k-inner (all tiles resident by now) so ACT/DVE
                # evacuation overlaps the PE stream group by group
                for mt in range(MT):
                    for lc in range(2):
                        psk = apsum.tile([128, 512], f32,
                                         name=f"ps_k{mt}_{lc}", tag="aps")
                        for k in range(KT):
                            nc.tensor.matmul(
                                psk,
                                wk_sb[:, k, 128 * mt:128 * (mt + 1)],
                                hid_sb[:, k, 512 * lc:512 * (lc + 1)],
                                start=(k == 0), stop=(k == KT - 1))
                        nc.scalar.activation(
                            out=kT_sb[:, mt, 512 * lc:512 * (lc + 1)],
                            in_=psk, func=Act.Identity,
                            bias=bk_sb[:, mt:mt + 1], scale=1.0)

                bvb4 = bvb_sb.rearrange("p (a s e) -> p a s e", a=4, s=2, e=64)
                for lt in range(LT):
                    psv = apsum.tile([128, 512], f32, name=f"ps_v{lt}",
                                     tag="aps")
                    for k in range(KT):
                        nc.tensor.matmul(psv,
                                         hid_sb[:, k, 128 * lt:128 * (lt + 1)],
                                         wv_sb[:, k, :],
                                         start=(k == 0), stop=(k == KT - 1))
                    dst = vaug[:, lt, :, :].rearrange(
                        "p a (s e) -> p a s e", s=2, e=65)[:, :, :, 0:64]
                    nc.vector.tensor_tensor(
                        dst, psv.rearrange("p (a s e) -> p a s e",
                                           a=4, s=2, e=64),
                        bvb4, op=AluOp.add)

            # ---------------- Stage B: attention ----------------
            with tc.tile_pool(name="bands", bufs=3) as bandpool, \
                 tc.tile_pool(name="kdp", bufs=4) as kdpool, \
                 tc.tile_pool(name="b1np", bufs=4) as b1npool, \
                 tc.tile_pool(name="b2tp", bufs=4) as b2tpool, \
                 tc.tile_pool(name="expp", bufs=4) as expool, \
                 tc.tile_pool(name="ctxtp", bufs=8) as ctxTpool, \
                 tc.tile_pool(name="rsp", bufs=4) as rspool, \
                 tc.tile_pool(name="ctxop", bufs=4) as ctxopool, \
                 tc.tile_pool(name="ps512", bufs=6, space="PSUM") as ps512, \
                 tc.tile_pool(name="ctxpsp", bufs=2, space="PSUM") as ctxpsp:

                def phase1(pair):
                    """Qrev bands for both head-halves of the pair."""
                    qrev = []
                    for hs in range(2):
                        qrev.append(bandpool.tile([128, LT, BAND], bf16,
                                                  name=f"qrev{pair}_{hs}",
                                                  tag="qrev"))
                    for i in range(LT):
                        for c0, cw in CHUNKS:
                            for hs in range(2):
                                hp = slice(64 * hs, 64 * (hs + 1))
                                psq = ps512.tile([128, cw], f32,
                                                 name=f"psq{pair}_{i}_{hs}_{c0}",
                                                 tag="ps")
                                nc.tensor.matmul(
                                    psq,
                                    qT_sb[hp, pair, 128 * i:128 * (i + 1)],
                                    ERV_sb[hp, _t0(i) + c0:_t0(i) + c0 + cw],
                                    start=True, stop=True)
                                if hs:
                                    nc.scalar.copy(qrev[hs][:, i, c0:c0 + cw],
                                                   psq)
                                else:
                                    nc.vector.tensor_copy(
                                        qrev[hs][:, i, c0:c0 + cw], psq)
                    return qrev

                def issue_b1n(pair, hs, rtp, qrev_t):
                    t = b1npool.tile([128, LT, 256], bf16,
                                     name=f"b1n{pair}_{hs}_{rtp}", tag="b1n")
                    src = bass.AP(
                        tensor=qrev_t.tensor,
                        offset=256 * rtp + 127,
                        ap=[[LT * BAND - 1, 128], [BAND, LT], [1, 256]])
                    nc.sync.dma_start(out=t, in_=src)
                    return t

                def kd_bands(pair, hs, rt):
                    hp = slice(64 * hs, 64 * (hs + 1))
                    kdt = kdpool.tile([128, BAND], bf16,
                                      name=f"kd{pair}_{hs}_{rt}", tag="kd")
                    for c0, cw in CHUNKS:
                        psk = ps512.tile([128, cw], f32,
                                         name=f"psk{pair}_{hs}_{rt}_{c0}",
                                         tag="ps")
                        nc.tensor.matmul(
                            psk,
                            kT_sb[hp, pair, 128 * rt:128 * (rt + 1)],
                            ET_sb[hp, _t0(rt) + c0:_t0(rt) + c0 + cw],
                            start=True, stop=True)
                        nc.vector.tensor_copy(kdt[:, c0:c0 + cw], psk)
                    t = b2tpool.tile([128, 2, 512], bf16,
                                     name=f"b2t{pair}_{hs}_{rt}", tag="b2t")
                    nc.sync.dma_start(
                        out=t,
                        in_=bass.AP(tensor=kdt.tensor, offset=127,
                                    ap=[[BAND - 1, 128], [512, 2], [1, 512]]))
                    return t

                def prologue(pair, hs, qrev_t):
                    """kd bands + first b1n square block for the next pass."""
                    return {
                        "b2t": {0: kd_bands(pair, hs, 0),
                                1: kd_bands(pair, hs, 1)},
                        "b1n0": issue_b1n(pair, hs, 0, qrev_t),
                    }

                def head_pass(pair, hs, qrev_t, prelude, nxt):
                    """Full attention for one head-half of a pair.  Consumes
                    rt 0/1 kd+b1n state from `prelude`; in its tail slots,
                    builds the same for the next pass `nxt`."""
                    hp = slice(64 * hs, 64 * (hs + 1))
                    b1n = {0: prelude["b1n0"]}
                    b2t = dict(prelude["b2t"])
                    next_prelude = {"b2t": {}}

                    ctxps = {}
                    for lc in range(2):
                        ctxps[lc] = ctxpsp.tile([65, 512], f32,
                                                name=f"ctxps{pair}_{hs}_{lc}",
                                                tag="ctxps")

                    ex = {}

                    def scores(rt):
                        for lc in range(2):
                            pss = ps512.tile([128, 512], f32,
                                             name=f"pss{pair}_{hs}_{rt}_{lc}",
                                             tag="ps")
                            nc.tensor.matmul(
                                pss,
                                kT_sb[hp, pair, 128 * rt:128 * (rt + 1)],
                                qT_sb[hp, pair, 512 * lc:512 * (lc + 1)],
                                start=True, stop=False)
                            # Q-side bias: pss[:, 128s:...] += b1n_sq^T @ I
                            h = 128 * (rt % 2)
                            for s in range(4):
                                sq = b1n[rt // 2][:, 4 * lc + s, h:h + 128]
                                nc.tensor.matmul(
                                    pss[:, 128 * s:128 * (s + 1)],
                                    sq, ident_bf,
                                    start=False, stop=False,
                                    skip_group_check=True)
                            # K-side bias via identity matmul
                            nc.tensor.matmul(
                                pss, ident_bf, b2t[rt][:, lc, :],
                                start=False, stop=True, skip_group_check=True)
                            e = expool.tile([128, 512], bf16,
                                            name=f"ex{pair}_{hs}_{rt}_{lc}",
                                            tag="ex")
                            nc.scalar.activation(out=e, in_=pss, func=Act.Exp,
                                                 scale=0.125)
                            ex[(rt, lc)] = e
                        del b2t[rt]

                    def pv(rt):
                        for lc in range(2):
                            nc.tensor.matmul(
                                ctxps[lc],
                                vaug[:, rt, pair, 65 * hs:65 * (hs + 1)],
                                ex.pop((rt, lc)),
                                start=(rt == 0), stop=(rt == LT - 1))

                    # software-pipelined rt loop; kd runs 2 ahead of scores,
                    # pv 1 behind; tail slots prefetch the next pass
                    for it in range(LT + 1):
                        if it in (0, 2, 4):
                            b1n[it // 2 + 1] = issue_b1n(pair, hs, it // 2 + 1,
                                                         qrev_t)
                        rtk = it + 2
                        if rtk < LT:
                            b2t[rtk] = kd_bands(pair, hs, rtk)
                        elif nxt is not None and rtk - LT <= 1:
                            np_, nh, nq = nxt
                            next_prelude["b2t"][rtk - LT] = kd_bands(
                                np_, nh, rtk - LT)
                            if rtk - LT == 1:
                                next_prelude["b1n0"] = issue_b1n(
                                    np_, nh, 0, nq)
                        if it < LT:
                            scores(it)
                        if it >= 1:
                            pv(it - 1)

                    # evacuate context accumulators
                    ctxT = {}
                    for lc in range(2):
                        cT = ctxTpool.tile([65, 512], bf16,
                                           name=f"ctxT{pair}_{hs}_{lc}",
                                           tag="ctxT")
                        nc.scalar.copy(cT, ctxps[lc])
                        ctxT[lc] = cT
                    return ctxT, next_prelude

                def phase3(pair, ctxT):
                    """Transpose ctx back to [l, d] (sums ride along as the
                    65th column), normalize, store with one batched DMA."""
                    ctxo = ctxopool.tile([128, LT, 128], f32,
                                         name=f"ctxo{pair}", tag="ctxo")
                    for i in range(LT):
                        lc, s = divmod(i, 4)
                        # hs blocks at 68-col stride keep psum APs 8B-aligned
                        ctp = ps512.tile([128, 136], f32,
                                         name=f"ctp{pair}_{i}", tag="ps")
                        for hs in range(2):
                            nc.tensor.matmul(
                                ctp[:, 68 * hs:68 * hs + 65],
                                ctxT[hs][lc][0:65, 128 * s:128 * (s + 1)],
                                ident_bf[0:65, 0:65],
                                start=True, stop=True, skip_group_check=True)
                        rs = rspool.tile([128, 2], f32,
                                         name=f"rs{pair}_{i}", tag="rs")
                        for hs in range(2):
                            nc.vector.reciprocal(
                                rs[:, hs:hs + 1],
                                ctp[:, 68 * hs + 64:68 * hs + 65])
                        for hs in range(2):
                            nc.scalar.activation(
                                out=ctxo[:, i, 64 * hs:64 * (hs + 1)],
                                in_=ctp[:, 68 * hs:68 * hs + 64],
                                func=Act.Copy, scale=rs[:, hs:hs + 1])
                    dst = bass.AP(tensor=outd, offset=128 * pair,
                                  ap=[[512, 128], [512 * 128, LT], [1, 128]])
                    nc.sync.dma_start(out=dst, in_=ctxo)

                qrev = phase1(0)
                prelude = prologue(0, 0, qrev[0])
                for pair in range(MT):
                    ctxT = {}
                    ctxT[0], prelude = head_pass(
                        pair, 0, qrev[0], prelude, (pair, 1, qrev[1]))
                    qrev_next = phase1(pair + 1) if pair + 1 < MT else None
                    nxt = ((pair + 1, 0, qrev_next[0])
                           if qrev_next is not None else None)
                    ctxT[1], prelude = head_pass(
                        pair, 1, qrev[1], prelude, nxt)
                    phase3(pair, ctxT)
                    qrev = qrev_next

    nc.compile()
    return nc


def _get_program():
    global _PROGRAM
    if _PROGRAM is None:
        _PROGRAM = _build_program()
    return _PROGRAM


def kernel(hidden_states, attention_mask, Wq, bq, Wk, bk, Wv, bv, dist_emb):
    global _LAST_RESULTS
    import ml_dtypes
    from concourse.bass_utils import run_bass_kernel_spmd

    bfloat16 = ml_dtypes.bfloat16
    hsv = np.asarray(hidden_states, dtype=np.float32)
    Wqv = np.asarray(Wq, dtype=np.float32)
    Wkv = np.asarray(Wk, dtype=np.float32)
    Wvv = np.asarray(Wv, dtype=np.float32)
    bqv = np.asarray(bq, dtype=np.float32)
    bkv = np.asarray(bk, dtype=np.float32)
    bvv = np.asarray(bv, dtype=np.float32)
    Ev = np.asarray(dist_emb, dtype=np.float32)

    ET = np.zeros((64, ETW), np.float32)
    ET[:, :2047] = Ev.T
    ET2 = np.ascontiguousarray(np.concatenate([ET, ET], axis=0)).astype(bfloat16)
    ERV = np.zeros((64, ETW), np.float32)
    ERV[:, :2047] = Ev[::-1].T
    ERV2 = np.ascontiguousarray(np.concatenate([ERV, ERV], axis=0)).astype(bfloat16)

    in_maps = []
    for c in range(8):
        b, hh = divmod(c, 2)
        sl = slice(512 * hh, 512 * (hh + 1))
        in_maps.append({
            "hidT": np.ascontiguousarray(hsv[b].T).astype(bfloat16),
            "wqT": np.ascontiguousarray(Wqv[sl].T).astype(bfloat16),
            "wkT": np.ascontiguousarray(Wkv[sl].T).astype(bfloat16),
            "wvT": np.ascontiguousarray(Wvv[sl].T).astype(bfloat16),
            "bq2": np.ascontiguousarray(bqv[sl].reshape(MT, 128).T),
            "bk2": np.ascontiguousarray(bkv[sl].reshape(MT, 128).T),
            "bvb": np.ascontiguousarray(np.tile(bvv[sl][None, :], (128, 1))),
            "ETd": ET2,
            "ERVd": ERV2,
        })

    nc = _get_program()
    res = run_bass_kernel_spmd(nc, in_maps, core_ids=list(range(8)))
    _LAST_RESULTS = res

    out = np.zeros((4, L, DM), np.float32)
    for c in range(8):
        b, hh = divmod(c, 2)
        out[b, :, 512 * hh:512 * (hh + 1)] = res.results[c]["out"]
    return out


# revision 16
# speedup vs baseline: 1.0566x; 1.0566x over previous
"""BertSelfAttention (relative_key_query position embeddings) on 8 TRN2 cores.

Problem: B=4, L=1024, DM=1024, H=16, D=64, MAXPOS=1024.
  q/k/v = hidden @ W.T + b  (per-head split)
  scores = q k^T + einsum(q, pe) + einsum(k, pe);  pe[l,r] = dist_emb[l-r+1023]
  probs = softmax(scores/8);  out = probs @ v

Sharding: core c handles batch b = c//2 and 8 heads (half) hh = c%2.
Per core the computation runs in "transposed score" orientation
(scoresT[r, l]): the PV matmul is transpose-free and the K-side
relative-position bias is a per-partition-shifted ("skewed") DMA from
Kd = k @ E^T.  The Q-side bias is extracted with the same skewed DMA in
natural orientation from Qrev = q @ Erev^T and folded into the score PSUM
with regular bf16 matmuls against an identity (out += b1n^T @ I).
Softmax uses no max-subtraction (scores are O(1) by construction) and
normalization is deferred past the PV matmul via an appended ones-column
on V; the row-sums ride along the final context transpose as a 65th
column so no scatter-DMAs are needed.

All matmul operands are bf16 (fp32 PSUM accumulation); phase 2 runs one
head-half at a time so 6 score/band PSUM banks + 2 context accumulators
fit the 8-bank budget with deep software pipelining (kd bands one r-tile
ahead, PV one r-tile behind) to keep the PE warm.
"""
import os
import sys

import numpy as np

if "/opt/trn_rl_repo" not in sys.path:
    sys.path.insert(0, "/opt/trn_rl_repo")

_PROGRAM = None
_LAST_RESULTS = None

# ---- fixed shapes ----
L = 1024          # sequence length
DM = 1024         # model dim
NH = 8            # heads per core
D = 64            # head dim
MT = 4            # m-tiles (head pairs) per core
KT = 8            # dm contraction tiles
LT = 8            # l (and r) tiles of 128
BAND = 1152       # stored band width per 128-row tile
ETW = 2048        # padded dist-emb width
CHUNKS = ((0, 512), (512, 512), (1024, 128))  # band psum chunking


def _t0(j):
    # band start for row-tile j: t = l - r + 1023 over full opposite dim
    return 896 - 128 * j


def _build_program():
    import concourse.bass as bass
    from concourse import bacc
    import concourse.mybir as mybir
    import concourse.tile as tile
    from concourse.masks import make_identity

    f32 = mybir.dt.float32
    bf16 = mybir.dt.bfloat16
    AluOp = mybir.AluOpType
    Act = mybir.ActivationFunctionType

    nc = bacc.Bacc("TRN2", target_bir_lowering=False, debug=False)

    hidT = nc.dram_tensor("hidT", [DM, L], bf16, kind="ExternalInput")
    wqT = nc.dram_tensor("wqT", [DM, 512], bf16, kind="ExternalInput")
    wkT = nc.dram_tensor("wkT", [DM, 512], bf16, kind="ExternalInput")
    wvT = nc.dram_tensor("wvT", [DM, 512], bf16, kind="ExternalInput")
    bq2 = nc.dram_tensor("bq2", [128, MT], f32, kind="ExternalInput")
    bk2 = nc.dram_tensor("bk2", [128, MT], f32, kind="ExternalInput")
    bvb = nc.dram_tensor("bvb", [128, 512], f32, kind="ExternalInput")
    ETd = nc.dram_tensor("ETd", [128, ETW], bf16, kind="ExternalInput")
    ERVd = nc.dram_tensor("ERVd", [128, ETW], bf16, kind="ExternalInput")
    outd = nc.dram_tensor("out", [L, 512], f32, kind="ExternalOutput")

    with tile.TileContext(nc) as tc:
        import contextlib
        stack = contextlib.ExitStack()
        with stack:
            persist = stack.enter_context(tc.tile_pool(name="persist", bufs=1))

            qT_sb = persist.tile([128, MT, L], bf16, name="qT_sb")
            kT_sb = persist.tile([128, MT, L], bf16, name="kT_sb")
            vaug = persist.tile([128, LT, MT, 130], bf16, name="vaug")
            ET_sb = persist.tile([128, ETW], bf16, name="ET_sb")
            ERV_sb = persist.tile([128, ETW], bf16, name="ERV_sb")
            ident32 = persist.tile([128, 128], f32, name="ident32")
            ident_bf = persist.tile([128, 128], bf16, name="ident_bf")
            bq_sb = persist.tile([128, MT], f32, name="bq_sb")
            bk_sb = persist.tile([128, MT], f32, name="bk_sb")
            bvb_sb = persist.tile([128, 512], f32, name="bvb_sb")

            # small/replicated tensors on the gpsimd (SWDGE) ring so the two
            # HWDGE rings are free for hid (sync) and weights (scalar)
            nc.gpsimd.dma_start(out=ET_sb, in_=ETd[:, :])
            nc.gpsimd.dma_start(out=ERV_sb, in_=ERVd[:, :])
            nc.gpsimd.dma_start(out=bq_sb, in_=bq2[:, :])
            nc.gpsimd.dma_start(out=bk_sb, in_=bk2[:, :])
            nc.gpsimd.dma_start(out=bvb_sb, in_=bvb[:, :])
            make_identity(nc, ident32)
            nc.vector.tensor_copy(ident_bf, ident32)

            # ones columns of vaug (col 64 and 129 of each [*,130] block)
            nc.vector.memset(vaug[:, :, :, 64:65], 1.0)
            nc.vector.memset(vaug[:, :, :, 129:130], 1.0)

            # ---------------- Stage A: projections ----------------
            with tc.tile_pool(name="stagea", bufs=1) as apool, \
                 tc.tile_pool(name="wpool", bufs=2) as wpool, \
                 tc.tile_pool(name="apsum", bufs=8, space="PSUM") as apsum:
                hid_sb = apool.tile([128, KT, L], bf16, name="hid_sb")
                for k in range(KT):
                    nc.sync.dma_start(out=hid_sb[:, k, :],
                                      in_=hidT[128 * k:128 * (k + 1), :])

                wq_sb = wpool.tile([128, KT, 512], bf16, name="wq_sb", tag="w")
                wk_sb = wpool.tile([128, KT, 512], bf16, name="wk_sb", tag="wk")
                wv_sb = wpool.tile([128, KT, 512], bf16, name="wv_sb", tag="wv")
                for w_sb, wdram in ((wq_sb, wqT), (wk_sb, wkT), (wv_sb, wvT)):
                    for k in range(KT):
                        nc.scalar.dma_start(out=w_sb[:, k, :],
                                            in_=wdram[128 * k:128 * (k + 1), :])

                # q: k-outer with 8 live accumulators — matmuls start as soon
                # as the first weight k-tile lands
                ps = {}
                for mt in range(MT):
                    for lc in range(2):
                        ps[(mt, lc)] = apsum.tile([128, 512], f32,
                                                  name=f"ps_q{mt}_{lc}",
                                                  tag="aps")
                for k in range(KT):
                    for mt in range(MT):
                        for lc in range(2):
                            nc.tensor.matmul(
                                ps[(mt, lc)],
                                wq_sb[:, k, 128 * mt:128 * (mt + 1)],
                                hid_sb[:, k, 512 * lc:512 * (lc + 1)],
                                start=(k == 0), stop=(k == KT - 1))
                for mt in range(MT):
                    for lc in range(2):
                        nc.scalar.activation(
                            out=qT_sb[:, mt, 512 * lc:512 * (lc + 1)],
                            in_=ps[(mt, lc)], func=Act.Identity,
                            bias=bq_sb[:, mt:mt + 1], scale=1.0)

                # k/v: k-inner (all tiles resident by now) so ACT/DVE
                # evacuation overlaps the PE stream group by group
                for mt in range(MT):
                    for lc in range(2):
                        psk = apsum.tile([128, 512], f32,
                                         name=f"ps_k{mt}_{lc}", tag="aps")
                        for k in range(KT):
                            nc.tensor.matmul(
                                psk,
                                wk_sb[:, k, 128 * mt:128 * (mt + 1)],
                                hid_sb[:, k, 512 * lc:512 * (lc + 1)],
                                start=(k == 0), stop=(k == KT - 1))
                        nc.scalar.activation(
                            out=kT_sb[:, mt, 512 * lc:512 * (lc + 1)],
                            in_=psk, func=Act.Identity,
                            bias=bk_sb[:, mt:mt + 1], scale=1.0)

                bvb4 = bvb_sb.rearrange("p (a s e) -> p a s e", a=4, s=2, e=64)
                for lt in range(LT):
                    psv = apsum.tile([128, 512], f32, name=f"ps_v{lt}",
                                     tag="aps")
                    for k in range(KT):
                        nc.tensor.matmul(psv,
                                         hid_sb[:, k, 128 * lt:128 * (lt + 1)],
                                         wv_sb[:, k, :],
                                         start=(k == 0), stop=(k == KT - 1))
                    dst = vaug[:, lt, :, :].rearrange(
                        "p a (s e) -> p a s e", s=2, e=65)[:, :, :, 0:64]
                    nc.vector.tensor_tensor(
                        dst, psv.rearrange("p (a s e) -> p a s e",
                                           a=4, s=2, e=64),
                        bvb4, op=AluOp.add)

            # ---------------- Stage B: attention ----------------
            with tc.tile_pool(name="bands", bufs=2) as bandpool, \
                 tc.tile_pool(name="kdp", bufs=4) as kdpool, \
                 tc.tile_pool(name="b1np", bufs=4) as b1npool, \
                 tc.tile_pool(name="b2tp", bufs=4) as b2tpool, \
                 tc.tile_pool(name="expp", bufs=4) as expool, \
                 tc.tile_pool(name="ctxtp", bufs=8) as ctxTpool, \
                 tc.tile_pool(name="rsp", bufs=4) as rspool, \
                 tc.tile_pool(name="ctxop", bufs=2) as ctxopool, \
                 tc.tile_pool(name="kdps", bufs=3, space="PSUM") as kdps, \
                 tc.tile_pool(name="pssp", bufs=3, space="PSUM") as pssp, \
                 tc.tile_pool(name="ctxpsp", bufs=2, space="PSUM") as ctxpsp:

                def qrev_steps(pair, hs):
                    """Band tile + one thunk per chunk matmul+copy, to be
                    interleaved into the preceding head pass."""
                    qrev_t = bandpool.tile([128, LT, BAND], bf16,
                                           name=f"qrev{pair}_{hs}", tag="qrev")
                    hp = slice(64 * hs, 64 * (hs + 1))
                    steps = []
                    for i in range(LT):
                        for ci, (c0, cw) in enumerate(CHUNKS):
                            def go(i=i, ci=ci, c0=c0, cw=cw):
                                psq = kdps.tile([128, cw], f32,
                                                name=f"psq{pair}_{hs}_{i}_{ci}",
                                                tag="kp")
                                nc.tensor.matmul(
                                    psq,
                                    qT_sb[hp, pair, 128 * i:128 * (i + 1)],
                                    ERV_sb[hp, _t0(i) + c0:_t0(i) + c0 + cw],
                                    start=True, stop=True)
                                dst = qrev_t[:, i, c0:c0 + cw]
                                if (i + ci) % 2:
                                    nc.scalar.copy(dst, psq)
                                else:
                                    nc.vector.tensor_copy(dst, psq)
                            steps.append(go)
                    return qrev_t, steps

                def issue_b1n(pair, hs, rtp, qrev_t):
                    t = b1npool.tile([128, LT, 256], bf16,
                                     name=f"b1n{pair}_{hs}_{rtp}", tag="b1n")
                    src = bass.AP(
                        tensor=qrev_t.tensor,
                        offset=256 * rtp + 127,
                        ap=[[LT * BAND - 1, 128], [BAND, LT], [1, 256]])
                    nc.sync.dma_start(out=t, in_=src)
                    return t

                def kd_bands(pair, hs, rt):
                    hp = slice(64 * hs, 64 * (hs + 1))
                    kdt = kdpool.tile([128, BAND], bf16,
                                      name=f"kd{pair}_{hs}_{rt}", tag="kd")
                    for c0, cw in CHUNKS:
                        psk = kdps.tile([128, cw], f32,
                                        name=f"psk{pair}_{hs}_{rt}_{c0}",
                                        tag="kp")
                        nc.tensor.matmul(
                            psk,
                            kT_sb[hp, pair, 128 * rt:128 * (rt + 1)],
                            ET_sb[hp, _t0(rt) + c0:_t0(rt) + c0 + cw],
                            start=True, stop=True)
                        nc.vector.tensor_copy(kdt[:, c0:c0 + cw], psk)
                    t = b2tpool.tile([128, 2, 512], bf16,
                                     name=f"b2t{pair}_{hs}_{rt}", tag="b2t")
                    nc.sync.dma_start(
                        out=t,
                        in_=bass.AP(tensor=kdt.tensor, offset=127,
                                    ap=[[BAND - 1, 128], [512, 2], [1, 512]]))
                    return t

                def prologue(pair, hs, qrev_t):
                    """kd bands + first b1n square block for the next pass."""
                    return {
                        "b2t": {0: kd_bands(pair, hs, 0),
                                1: kd_bands(pair, hs, 1)},
                        "b1n0": issue_b1n(pair, hs, 0, qrev_t),
                    }

                def head_pass(pair, hs, qrev_t, prelude, nxt, extra=()):
                    """Full attention for one head-half of a pair.  Consumes
                    rt 0/1 kd+b1n state from `prelude`; in its tail slots,
                    builds the same for the next pass `nxt`.  `extra` thunks
                    (next-pass qrev chunks, prev-pair phase3 pieces) are
                    drained a few per iteration to keep the PE stream and the
                    copy engines uniformly loaded."""
                    hp = slice(64 * hs, 64 * (hs + 1))
                    b1n = {0: prelude["b1n0"]}
                    b2t = dict(prelude["b2t"])
                    next_prelude = {"b2t": {}}
                    extra = list(extra)

                    ctxps = {}
                    for lc in range(2):
                        ctxps[lc] = ctxpsp.tile([65, 512], f32,
                                                name=f"ctxps{pair}_{hs}_{lc}",
                                                tag="ctxps")

                    ex = {}

                    def scores(rt):
                        for lc in range(2):
                            pss = pssp.tile([128, 512], f32,
                                            name=f"pss{pair}_{hs}_{rt}_{lc}",
                                            tag="ps")
                            nc.tensor.matmul(
                                pss,
                                kT_sb[hp, pair, 128 * rt:128 * (rt + 1)],
                                qT_sb[hp, pair, 512 * lc:512 * (lc + 1)],
                                start=True, stop=False)
                            # Q-side bias: pss[:, 128s:...] += b1n_sq^T @ I
                            h = 128 * (rt % 2)
                            for s in range(4):
                                sq = b1n[rt // 2][:, 4 * lc + s, h:h + 128]
                                nc.tensor.matmul(
                                    pss[:, 128 * s:128 * (s + 1)],
                                    sq, ident_bf,
                                    start=False, stop=False,
                                    skip_group_check=True)
                            # K-side bias via identity matmul
                            nc.tensor.matmul(
                                pss, ident_bf, b2t[rt][:, lc, :],
                                start=False, stop=True, skip_group_check=True)
                            e = expool.tile([128, 512], bf16,
                                            name=f"ex{pair}_{hs}_{rt}_{lc}",
                                            tag="ex")
                            nc.scalar.activation(out=e, in_=pss, func=Act.Exp,
                                                 scale=0.125)
                            ex[(rt, lc)] = e
                        del b2t[rt]

                    def pv(rt):
                        for lc in range(2):
                            nc.tensor.matmul(
                                ctxps[lc],
                                vaug[:, rt, pair, 65 * hs:65 * (hs + 1)],
                                ex.pop((rt, lc)),
                                start=(rt == 0), stop=(rt == LT - 1))

                    # software-pipelined rt loop; kd runs 2 ahead of scores,
                    # pv 1 behind; tail slots prefetch the next pass
                    for it in range(LT + 1):
                        for _ in range(4):
                            if extra:
                                extra.pop(0)()
                        if it in (0, 2, 4):
                            b1n[it // 2 + 1] = issue_b1n(pair, hs, it // 2 + 1,
                                                         qrev_t)
                        rtk = it + 2
                        if rtk < LT:
                            b2t[rtk] = kd_bands(pair, hs, rtk)
                        elif nxt is not None and rtk - LT <= 1:
                            np_, nh, nq = nxt
                            next_prelude["b2t"][rtk - LT] = kd_bands(
                                np_, nh, rtk - LT)
                            if rtk - LT == 1:
                                next_prelude["b1n0"] = issue_b1n(
                                    np_, nh, 0, nq)
                        if it < LT:
                            scores(it)
                        if it >= 1:
                            pv(it - 1)
                    while extra:
                        extra.pop(0)()

                    # evacuate context accumulators
                    ctxT = {}
                    for lc in range(2):
                        cT = ctxTpool.tile([65, 512], bf16,
                                           name=f"ctxT{pair}_{hs}_{lc}",
                                           tag="ctxT")
                        nc.scalar.copy(cT, ctxps[lc])
                        ctxT[lc] = cT
                    return ctxT, next_prelude

                def phase3_steps(pair, ctxT):
                    """Transpose ctx back to [l, d] (sums ride along as the
                    65th column), normalize, store with one batched DMA."""
                    ctxo = ctxopool.tile([128, LT, 128], f32,
                                         name=f"ctxo{pair}", tag="ctxo")

                    def step(i):
                        lc, s = divmod(i, 4)
                        # hs blocks at 68-col stride keep psum APs 8B-aligned
                        ctp = kdps.tile([128, 136], f32,
                                        name=f"ctp{pair}_{i}", tag="kp")
                        for hs in range(2):
                            nc.tensor.matmul(
                                ctp[:, 68 * hs:68 * hs + 65],
                                ctxT[hs][lc][0:65, 128 * s:128 * (s + 1)],
                                ident_bf[0:65, 0:65],
                                start=True, stop=True, skip_group_check=True)
                        rs = rspool.tile([128, 2], f32,
                                         name=f"rs{pair}_{i}", tag="rs")
                        for hs in range(2):
                            nc.vector.reciprocal(
                                rs[:, hs:hs + 1],
                                ctp[:, 68 * hs + 64:68 * hs + 65])
                        for hs in range(2):
                            nc.vector.tensor_scalar_mul(
                                ctxo[:, i, 64 * hs:64 * (hs + 1)],
                                ctp[:, 68 * hs:68 * hs + 64],
                                rs[:, hs:hs + 1])
                        if i == LT - 1:
                            dst = bass.AP(
                                tensor=outd, offset=128 * pair,
                                ap=[[512, 128], [512 * 128, LT], [1, 128]])
                            nc.sync.dma_start(out=dst, in_=ctxo)

                    return [lambda i=i: step(i) for i in range(LT)]

                # prologue: first qrev band pair computed inline
                qrev0, steps00 = qrev_steps(0, 0)
                for s in steps00:
                    s()
                qrev1, steps01 = qrev_steps(0, 1)
                prelude = prologue(0, 0, qrev0)
                ctxT_prev = None
                qr = {0: qrev0, 1: qrev1}
                st = {1: steps01}
                for pair in range(MT):
                    extra0 = list(st.pop(1))
                    if ctxT_prev is not None:
                        extra0 += phase3_steps(pair - 1, ctxT_prev)
                    ctxT = {}
                    ctxT[0], prelude = head_pass(
                        pair, 0, qr[0], prelude, (pair, 1, qr[1]), extra0)
                    if pair + 1 < MT:
                        qrev_n0, steps_n0 = qrev_steps(pair + 1, 0)
                        nxt = (pair + 1, 0, qrev_n0)
                        extra1 = steps_n0
                    else:
                        qrev_n0, nxt, extra1 = None, None, ()
                    ctxT[1], prelude = head_pass(
                        pair, 1, qr[1], prelude, nxt, extra1)
                    if pair + 1 < MT:
                        qrev_n1, steps_n1 = qrev_steps(pair + 1, 1)
                        qr = {0: qrev_n0, 1: qrev_n1}
                        st = {1: steps_n1}
                    ctxT_prev = ctxT
                # final pair's phase3 as tail
                for s in phase3_steps(MT - 1, ctxT_prev):
                    s()

    nc.compile()
    return nc


def _get_program():
    global _PROGRAM
    if _PROGRAM is None:
        _PROGRAM = _build_program()
    return _PROGRAM


def kernel(hidden_states, attention_mask, Wq, bq, Wk, bk, Wv, bv, dist_emb):
    global _LAST_RESULTS
    import ml_dtypes
    from concourse.bass_utils import run_bass_kernel_spmd

    bfloat16 = ml_dtypes.bfloat16
    hsv = np.asarray(hidden_states, dtype=np.float32)
    Wqv = np.asarray(Wq, dtype=np.float32)
    Wkv = np.asarray(Wk, dtype=np.float32)
    Wvv = np.asarray(Wv, dtype=np.float32)
    bqv = np.asarray(bq, dtype=np.float32)
    bkv = np.asarray(bk, dtype=np.float32)
    bvv = np.asarray(bv, dtype=np.float32)
    Ev = np.asarray(dist_emb, dtype=np.float32)

    ET = np.zeros((64, ETW), np.float32)
    ET[:, :2047] = Ev.T
    ET2 = np.ascontiguousarray(np.concatenate([ET, ET], axis=0)).astype(bfloat16)
    ERV = np.zeros((64, ETW), np.float32)
    ERV[:, :2047] = Ev[::-1].T
    ERV2 = np.ascontiguousarray(np.concatenate([ERV, ERV], axis=0)).astype(bfloat16)

    in_maps = []
    for c in range(8):
        b, hh = divmod(c, 2)
        sl = slice(512 * hh, 512 * (hh + 1))
        in_maps.append({
            "hidT": np.ascontiguousarray(hsv[b].T).astype(bfloat16),
            "wqT": np.ascontiguousarray(Wqv[sl].T).astype(bfloat16),
            "wkT": np.ascontiguousarray(Wkv[sl].T).astype(bfloat16),
            "wvT": np.ascontiguousarray(Wvv[sl].T).astype(bfloat16),
            "bq2": np.ascontiguousarray(bqv[sl].reshape(MT, 128).T),
            "bk2": np.ascontiguousarray(bkv[sl].reshape(MT, 128).T),
            "bvb": np.ascontiguousarray(np.tile(bvv[sl][None, :], (128, 1))),
            "ETd": ET2,
            "ERVd": ERV2,
        })

    nc = _get_program()
    res = run_bass_kernel_spmd(nc, in_maps, core_ids=list(range(8)))
    _LAST_RESULTS = res

    out = np.zeros((4, L, DM), np.float32)
    for c in range(8):
        b, hh = divmod(c, 2)
        out[b, :, 512 * hh:512 * (hh + 1)] = res.results[c]["out"]
    return out
